# revision 58
# baseline (speedup 1.0000x reference)
"""GCNConv forward on 8 Trainium2 NeuronCores (Bass/Tile), v5.

Strategy (graph/edge-cut parallelism):
  - Nodes padded to 102400 = 8 cores x 100 groups x 128; each core owns the
    scatter-sum for its 12800-node shard.
  - deg/norm precomputed host-side from edge_index (pure index bookkeeping,
    same bincount the stream-capacity prep already does); shipped as a
    [P, BPC] f32 input. x is shipped bf16 in a partition-blocked layout
    (contiguous >=1KB per-partition DMA descriptors dodge the sub-512B
    read-modify-write penalty).
  - Phase A: g = norm[src] * x (Pool-engine multiply, bf16) -> g_own ->
    ONE AllGather into a Shared-scratchpad g_full. Phase-A DMAs alternate
    the SP/ACT HWDGE queues and are issued first so the collective starts
    ~40us in; edge streams load during it (tile_wait_until defers e16/w).
  - Edge streams are split own/remote. Own-core-source edges (up to 128 per
    dst group, one tile per group) gather from g_own DURING the AllGather;
    spill + remote edges are grouped by (dst group of 128, src quarter
    q = src%4 over interleaved 25600-row tables) and gathered from g_full
    per (chunk of 5 groups, q) block with int16 idx dma_gathers.
  - One-hot dst masks in fp8e4 (0/1 exact, mixed-dtype matmul with bf16
    features verified exact) built per edge tile with tensor_scalar
    is_equal (iota bf16 vs [P,1] f32 slot scalar -> DVE fast path), issued
    ahead into a 384-deep pool so DVE pre-builds during the AllGather.
  - Scatter-sum via one-hot matmuls into per-group PSUM [P,128] (own tile
    first, then remote); evict bf16 (DVE), @W (PE), stage h bf16 + ssq
    (DVE mult+reduce); rl2 = exp(-0.5*ln(ssq+eps)) batched in 4 tail parts
    interleaved with phase C (minimal ACT table switches), tanh(h*rl2)
    fused via activation scale, f32 stores batched 4 groups per DMA.
"""

import numpy as np
import ml_dtypes

N, E, D = 100000, 625000, 128
P = 128
NCORES = 8
NPAD = 102400
SHARD = NPAD // NCORES        # 12800
BPC = SHARD // P              # 100 buckets (= dst groups) per core
GW = P                        # dst-group width = 128
NG = SHARD // GW              # 100 groups per core
GPC = 5                       # groups per chunk
NCHUNK = NG // GPC            # 20
NQ = 4                        # src quarter tables (interleaved: q = src % 4)
TQ = NPAD // NQ               # 25600 rows per quarter table
XCH = 10                      # buckets per phase-A x chunk
NPRE = 384                    # one-hot pool depth (prebuild during AllGather)

_CACHE = {}
_PREP_CACHE = {}


def _prep(edge_index):
    """Host-side partitioning (data movement / index bookkeeping only)."""
    src = edge_index[0].astype(np.int64)
    dst = edge_index[1].astype(np.int64)

    loops = np.arange(N, dtype=np.int64)
    src2 = np.concatenate([src, loops])
    dst2 = np.concatenate([dst, loops])
    core = dst2 // SHARD
    l = dst2 % SHARD
    gl = l // GW
    slot = l % GW

    # norm = deg^-0.5 from out-degree (incl self-loops); 0 for padding nodes
    deg = np.bincount(src2, minlength=NPAD).astype(np.float64)
    with np.errstate(divide="ignore"):
        norm = np.where(deg > 0, 1.0 / np.sqrt(deg), 0.0).astype(np.float32)
    nrm = norm.reshape(NCORES, BPC, P).transpose(0, 2, 1)  # [c, p, b]
    nrm = np.ascontiguousarray(nrm)

    # ---- own-core stream: up to 128 same-core-source edges per dst group,
    # gathered from g_own during the AllGather (one tile per group) ----
    score = src2 // SHARD
    is_own = score == core
    ocell = core * NG + gl
    oorder = np.lexsort((src2, np.where(is_own, ocell, 2**40)))
    n_own_all = int(is_own.sum())
    oo = oorder[:n_own_all]                      # own edges sorted by ocell
    oc_s = ocell[oo]
    ocounts = np.bincount(oc_s, minlength=NCORES * NG)
    ostarts = np.zeros(NCORES * NG + 1, np.int64)
    np.cumsum(ocounts, out=ostarts[1:])
    opos = np.arange(n_own_all) - ostarts[oc_s]
    keep = opos < P                              # first 128 per group
    okeep = oo[keep]
    oprt = opos[keep]
    og = gl[okeep]
    ocorek = core[okeep]
    olidx = (src2[okeep] % SHARD).astype(np.int16)
    # own idx stream: 10 chunks x 10 tiles, 16-row wrap, replicated x8
    o16 = np.zeros((NCORES, 16, NG * 8), np.int16)
    OCH = 10                                     # own groups per gather
    oib = (og % OCH) * P + oprt
    ocol16 = (og // OCH) * OCH * 8 + oib // 16
    orow16 = oib % 16
    o16[ocorek, orow16, ocol16] = olidx
    o16 = np.tile(o16, (1, 8, 1))
    odst = np.full((NCORES, P, NG), 999.0, np.float32)
    odst[ocorek, oprt, og] = slot[okeep]

    # ---- remote stream: everything not claimed by the own stream ----
    claimed = np.zeros(len(src2), bool)
    claimed[okeep] = True
    rmask = ~claimed
    src2 = src2[rmask]
    dst2 = dst2[rmask]
    core = core[rmask]
    gl = gl[rmask]
    slot = slot[rmask]
    q = src2 % NQ
    lidx = src2 // NQ

    cell = (core * NG + gl) * NQ + q
    order = np.lexsort((lidx, cell))
    cell_s = cell[order]
    counts = np.bincount(cell, minlength=NCORES * NG * NQ)
    starts = np.zeros(NCORES * NG * NQ + 1, np.int64)
    np.cumsum(counts, out=starts[1:])
    pos = np.arange(len(order)) - starts[cell_s]

    caps = np.ceil(
        counts.reshape(NCORES, NG, NQ).max(0) / P
    ).astype(np.int64)                                     # [NG, NQ]

    # single stream layout: (chunk, q, gl-in-chunk, t)
    tbase = np.zeros((NG, NQ), np.int64)
    blockstart = np.zeros((NCHUNK, NQ), np.int64)
    blockntiles = np.zeros((NCHUNK, NQ), np.int64)
    tc = 0
    for ch in range(NCHUNK):
        for qq in range(NQ):
            blockstart[ch, qq] = tc
            for gi in range(GPC):
                g = ch * GPC + gi
                tbase[g, qq] = tc
                tc += caps[g, qq]
            blockntiles[ch, qq] = tc - blockstart[ch, qq]
    totE = int(tc)

    ecore = core[order]
    egl = gl[order]
    eq = q[order]
    t = pos // P
    prt = pos % P

    gcol = tbase[egl, eq] + t
    chnk = egl // GPC
    ib = (gcol - blockstart[chnk, eq]) * P + prt
    col16 = blockstart[chnk, eq] * 8 + ib // 16
    row16 = ib % 16
    e16 = np.zeros((NCORES, 16, totE * 8), np.int16)
    e16[ecore, row16, col16] = lidx[order].astype(np.int16)
    e16 = np.tile(e16, (1, 8, 1))

    edst = np.full((NCORES, P, totE), 999.0, np.float32)
    edst[ecore, prt, gcol] = slot[order]

    return dict(
        e16=e16, edst=edst, nrm=nrm, o16=o16, odst=odst,
        caps=caps, tbase=tbase, blockstart=blockstart,
        blockntiles=blockntiles, totE=totE,
    )


def _build(prep):
    import concourse.bass as bass
    import concourse.bacc as bacc
    import concourse.mybir as mybir
    import concourse.tile as tile

    F32 = mybir.dt.float32
    BF16 = mybir.dt.bfloat16
    F8 = mybir.dt.float8e4
    I16 = mybir.dt.int16
    AF = mybir.ActivationFunctionType
    OP = mybir.AluOpType

    caps = prep["caps"]
    tbase = prep["tbase"]
    blockstart = prep["blockstart"]
    blockntiles = prep["blockntiles"]
    totE = prep["totE"]
    maxnt = int(blockntiles.max())

    nc = bacc.Bacc("TRN2", target_bir_lowering=False, debug=False)
    x_sh = nc.dram_tensor("x_sh", [P, BPC * D], BF16, kind="ExternalInput")
    w_in = nc.dram_tensor("w_in", [D, D], F32, kind="ExternalInput")
    iota_in = nc.dram_tensor("iota_in", [P, P], BF16, kind="ExternalInput")
    nrm_in = nc.dram_tensor("nrm_in", [P, BPC], F32, kind="ExternalInput")
    e16_in = nc.dram_tensor("e16_in", [P, totE * 8], I16, kind="ExternalInput")
    edst_in = nc.dram_tensor("edst_in", [P, totE], F32, kind="ExternalInput")
    o16_in = nc.dram_tensor("o16_in", [P, NG * 8], I16, kind="ExternalInput")
    odst_in = nc.dram_tensor("odst_in", [P, NG], F32, kind="ExternalInput")
    out = nc.dram_tensor("out", [SHARD, D], F32, kind="ExternalOutput")

    with tile.TileContext(nc) as tc:
        with (
            tc.tile_pool(name="const", bufs=1) as cst,
            tc.tile_pool(name="inp", bufs=1) as inp,
            tc.tile_pool(name="xp", bufs=3) as xp,
            tc.tile_pool(name="gp", bufs=3) as gp,
            tc.tile_pool(name="ohp", bufs=NPRE) as ohp,
            tc.tile_pool(name="oohp", bufs=NG) as oohp,
            tc.tile_pool(name="xgp", bufs=12) as xgp,
            tc.tile_pool(name="atp", bufs=8) as atp,
            tc.tile_pool(name="sqp", bufs=6) as sqp,
            tc.tile_pool(name="stfp", bufs=6) as stfp,
            tc.tile_pool(name="stage", bufs=1) as stg,
            tc.tile_pool(name="pagg", bufs=6, space="PSUM") as pap,
            tc.tile_pool(name="pw", bufs=2, space="PSUM") as pwp,
            tc.tile_pool(name="dram", bufs=1, space="DRAM") as drm,
        ):
            # ---- constants (phase-A-critical DMAs first) ----
            iota_t = cst.tile([P, P], BF16)
            w_sb = cst.tile([P, P], F32)
            w_bf = cst.tile([P, P], BF16)
            eps_t = cst.tile([P, 1], F32)
            nrm_t = inp.tile([P, BPC], F32)
            nc.sync.dma_start(out=nrm_t[:], in_=nrm_in[:])
            nc.gpsimd.memset(eps_t[:], 1e-30)

            # ---- staging ----
            out_stage = stg.tile([P, BPC * P], BF16)
            ssq = stg.tile([P, BPC], F32)
            rl2 = stg.tile([P, BPC], F32)

            g_own = drm.tile([SHARD, D], BF16)
            g_full = drm.tile([NPAD, D], BF16, addr_space="Shared")

            x_r = x_sh[:].rearrange("p (b f) -> p b f", f=D)
            gown_r = g_own[:].rearrange("(b p) f -> p b f", p=P)

            # ---- phase A: g = norm[src] * x (bf16) ----
            for xc in range(BPC // XCH):
                sl = slice(xc * XCH, (xc + 1) * XCH)
                eng = nc.sync if xc % 2 == 0 else nc.scalar
                xch = xp.tile([P, XCH, P], BF16, tag="xch")
                eng.dma_start(out=xch[:], in_=x_r[:, sl, :])
                gch = gp.tile([P, XCH, P], BF16, tag="gch")
                nc.gpsimd.tensor_tensor(
                    out=gch[:], in0=xch[:],
                    in1=nrm_t[:, sl].rearrange("p b -> p b ()")
                        .to_broadcast([P, XCH, P]),
                    op=OP.mult,
                )
                eng2 = nc.scalar if xc % 2 == 0 else nc.sync
                eng2.dma_start(out=gown_r[:, sl, :], in_=gch[:])

            nc.gpsimd.collective_compute(
                "AllGather",
                mybir.AluOpType.bypass,
                ins=[g_own.opt()],
                outs=[g_full.opt()],
                replica_groups=[list(range(NCORES))],
            )

            # ---- remaining input streams (load during the AllGather) ----
            e16_t = inp.tile([P, totE * 8], I16)
            edst_t = inp.tile([P, totE], F32)
            o16_t = inp.tile([P, NG * 8], I16)
            odst_t = inp.tile([P, NG], F32)
            nc.sync.dma_start(out=iota_t[:], in_=iota_in[:])
            nc.sync.dma_start(out=edst_t[:], in_=edst_in[:])
            nc.sync.dma_start(out=o16_t[:], in_=o16_in[:])
            nc.sync.dma_start(out=odst_t[:], in_=odst_in[:])

            # ---- one-hot builds (fp8; DVE runs ahead during the AllGather) ----
            ohs = {}
            for ch in range(NCHUNK):
                for qq in range(NQ):
                    for gi in range(GPC):
                        g = ch * GPC + gi
                        for t in range(int(caps[g, qq])):
                            col = int(tbase[g, qq]) + t
                            oh = ohp.tile([P, P], F8, tag="oh")
                            nc.vector.tensor_scalar(
                                out=oh[:], in0=iota_t[:],
                                scalar1=edst_t[:, col:col + 1], scalar2=None,
                                op0=OP.is_equal,
                            )
                            ohs[(g, qq, t)] = oh

            # ---- own-core stream: one-hots + gathers from g_own run during
            # the AllGather (no dependency on g_full) ----
            oohs = []
            for g in range(NG):
                ooh = oohp.tile([P, P], F8, tag="ooh")
                nc.vector.tensor_scalar(
                    out=ooh[:], in0=iota_t[:],
                    scalar1=odst_t[:, g:g + 1], scalar2=None,
                    op0=OP.is_equal,
                )
                oohs.append(ooh)
            xo_t = stg.tile([P, NG, P], BF16)
            OCH = 10
            for och in range(NG // OCH):
                nc.gpsimd.dma_gather(
                    out_ap=xo_t[:, och * OCH:(och + 1) * OCH, :],
                    in_ap=g_own[:],
                    idxs_ap=o16_t[:, och * OCH * 8:(och + 1) * OCH * 8],
                    num_idxs=OCH * P,
                    num_idxs_reg=OCH * P,
                    elem_size=P,
                    single_packet=False,
                )

            # e16/w deferred past phase A: load during the AllGather without
            # stealing phase-A DMA bandwidth
            with tc.tile_wait_until(0.09):
                nc.sync.dma_start(out=e16_t[:], in_=e16_in[:])
                nc.sync.dma_start(out=w_sb[:], in_=w_in[:])
                nc.vector.tensor_copy(w_bf[:], w_sb[:])

            # ---- phase C: gather + scatter-sum + @W + stage ----
            # tail parts fire as soon as their groups' ssq is complete
            TAILS = [(0, 40), (40, 64), (64, 84)]
            out_r = out[:].rearrange("(b p) f -> p b f", p=P)

            def tail_part(g0, g1):
                """rl2 + tanh + store for groups [g0, g1)."""
                sl2 = slice(g0, g1)
                nc.scalar.activation(rl2[:, sl2], ssq[:, sl2], AF.Ln,
                                     bias=eps_t[:])
                nc.scalar.activation(rl2[:, sl2], rl2[:, sl2], AF.Exp,
                                     scale=-0.5)
                for b0 in range(g0, g1, 4):
                    stf = stfp.tile([P, 4, P], F32, tag="stf")
                    for j in range(4):
                        g = b0 + j
                        nc.scalar.activation(
                            stf[:, j, :], out_stage[:, g * P:(g + 1) * P],
                            AF.Tanh, scale=rl2[:, g:g + 1],
                        )
                    nc.sync.dma_start(
                        out=out_r[:, b0:b0 + 4, :], in_=stf[:],
                    )

            for ch in range(NCHUNK):
                xgs = []
                for qq in range(NQ):
                    nt = int(blockntiles[ch, qq])
                    if nt == 0:
                        xgs.append(None)
                        continue
                    bs = int(blockstart[ch, qq])
                    xg = xgp.tile([P, maxnt, P], BF16, tag="xg")
                    gq = g_full[:].rearrange("(r s) f -> s r f", s=NQ)[qq]
                    nc.gpsimd.dma_gather(
                        out_ap=xg[:, :nt, :],
                        in_ap=gq,
                        idxs_ap=e16_t[:, bs * 8:(bs + nt) * 8],
                        num_idxs=nt * P,
                        num_idxs_reg=nt * P,
                        elem_size=P,
                        elem_step=NQ * P,
                        single_packet=False,
                    )
                    xgs.append(xg)
                for gi in range(GPC):
                    g = ch * GPC + gi
                    nmm = int(caps[g, :].sum()) + 1
                    pA = pap.tile([P, P], F32, space="PSUM")
                    nc.tensor.matmul(
                        pA[:], lhsT=xo_t[:, g, :], rhs=oohs[g][:],
                        start=True, stop=(nmm == 1),
                    )
                    mi = 1
                    for qq in range(NQ):
                        kq = int(caps[g, qq])
                        if kq == 0:
                            continue
                        p0 = int(tbase[g, qq]) - int(blockstart[ch, qq])
                        for t in range(kq):
                            nc.tensor.matmul(
                                pA[:],
                                lhsT=xgs[qq][:, p0 + t, :],
                                rhs=ohs.pop((g, qq, t))[:],
                                start=(mi == 0), stop=(mi == nmm - 1),
                            )
                            mi += 1
                    at = atp.tile([P, P], BF16, tag="at")
                    nc.vector.tensor_copy(at[:], pA[:])
                    pC = pwp.tile([P, P], F32, space="PSUM")
                    nc.tensor.matmul(
                        pC[:], lhsT=at[:], rhs=w_bf[:], start=True, stop=True,
                    )
                    ost = out_stage[:, g * P:(g + 1) * P]
                    nc.scalar.activation(ost, pC[:], AF.Copy)
                    sq = sqp.tile([P, P], BF16, tag="sq")
                    nc.vector.tensor_tensor(
                        out=sq[:], in0=ost, in1=ost, op=OP.mult,
                    )
                    nc.vector.tensor_reduce(
                        out=ssq[:, g:g + 1],
                        in_=sq[:].rearrange("p (a b) -> p a b", a=1),
                        axis=mybir.AxisListType.X, op=OP.add,
                    )
                done = (ch + 1) * GPC
                for t0, t1 in TAILS:
                    if done - GPC < t1 <= done:
                        tail_part(t0, t1)
            tail_part(TAILS[-1][1], NG)

    nc.compile()
    return nc


def _make_in_maps(x, W, prep):
    iota_row = np.tile(
        np.arange(P, dtype=np.float32), (P, 1)
    ).astype(ml_dtypes.bfloat16)
    x_pad = np.zeros((NPAD, D), ml_dtypes.bfloat16)
    x_pad[:N] = np.asarray(x, np.float32).astype(ml_dtypes.bfloat16)
    w_np = np.asarray(W, np.float32)
    in_maps = []
    for c in range(NCORES):
        in_maps.append({
            "x_sh": np.ascontiguousarray(
                x_pad[c * SHARD:(c + 1) * SHARD]
                .reshape(BPC, P, D).transpose(1, 0, 2).reshape(P, BPC * D)),
            "w_in": w_np,
            "iota_in": iota_row,
            "nrm_in": np.ascontiguousarray(prep["nrm"][c]),
            "e16_in": np.ascontiguousarray(prep["e16"][c]),
            "edst_in": np.ascontiguousarray(prep["edst"][c]),
            "o16_in": np.ascontiguousarray(prep["o16"][c]),
            "odst_in": np.ascontiguousarray(prep["odst"][c]),
        })
    return in_maps


def get_compiled(edge_index):
    ei = np.asarray(edge_index)
    pkey = hash(ei.tobytes())
    if pkey in _PREP_CACHE:
        prep = _PREP_CACHE[pkey]
    else:
        prep = _prep(ei)
        _PREP_CACHE.clear()
        _PREP_CACHE[pkey] = prep
    key = prep["caps"].tobytes()
    if key not in _CACHE:
        _CACHE[key] = _build(prep)
    return _CACHE[key], prep


def kernel(x, edge_index, W):
    from concourse.bass_utils import run_bass_kernel_spmd

    nc, prep = get_compiled(edge_index)
    in_maps = _make_in_maps(x, W, prep)
    res = run_bass_kernel_spmd(nc, in_maps, core_ids=list(range(NCORES)))
    big = np.concatenate([res.results[c]["out"] for c in range(NCORES)], axis=0)
    return np.ascontiguousarray(big[:N]).astype(np.float32)


# revision 68
# speedup vs baseline: 1.0163x; 1.0163x over previous
"""GCNConv forward on 8 Trainium2 NeuronCores (Bass/Tile), v5.

Strategy (graph/edge-cut parallelism):
  - Nodes padded to 102400 = 8 cores x 100 groups x 128; each core owns the
    scatter-sum for its 12800-node shard.
  - deg/norm precomputed host-side from edge_index (pure index bookkeeping,
    same bincount the stream-capacity prep already does); shipped as a
    [P, BPC] f32 input. x is shipped bf16 in a partition-blocked layout
    (contiguous >=1KB per-partition DMA descriptors dodge the sub-512B
    read-modify-write penalty).
  - Phase A: g = norm[src] * x (Pool-engine multiply, bf16) -> g_own ->
    ONE AllGather into a Shared-scratchpad g_full. Phase-A DMAs alternate
    the SP/ACT HWDGE queues and are issued first so the collective starts
    ~40us in; edge streams load during it (tile_wait_until defers e16/w).
  - Edge streams are split own/remote. Own-core-source edges (up to 128 per
    dst group, one tile per group) gather from g_own DURING the AllGather;
    spill + remote edges are grouped by (dst group of 128, src quarter
    q = src%4 over interleaved 25600-row tables) and gathered from g_full
    per (chunk, q) block with int16 idx dma_gathers; chunk sizes taper
    [5]x18+[4,3,2,1] so the end-of-kernel drain covers a 1-group pipeline.
  - One-hot dst masks in fp8e4 (0/1 exact, mixed-dtype matmul with bf16
    features verified exact) built per edge tile with tensor_scalar
    is_equal (iota bf16 vs [P,1] f32 slot scalar -> DVE fast path), issued
    ahead into a 384-deep pool so DVE pre-builds during the AllGather.
  - Scatter-sum via one-hot matmuls into per-group PSUM [P,128] (own tile
    first, then remote); evict bf16 (DVE), @W (PE), stage h bf16 + ssq
    (DVE mult+reduce); rl2 = exp(-0.5*ln(ssq+eps)) batched in 4 tail parts
    interleaved with phase C (minimal ACT table switches), tanh(h*rl2)
    fused via activation scale, f32 stores batched 4 groups per DMA.
"""

import numpy as np
import ml_dtypes

N, E, D = 100000, 625000, 128
P = 128
NCORES = 8
NPAD = 102400
SHARD = NPAD // NCORES        # 12800
BPC = SHARD // P              # 100 buckets (= dst groups) per core
GW = P                        # dst-group width = 128
NG = SHARD // GW              # 100 groups per core
CHS = [5] * 18 + [4, 3, 2, 1]  # groups per chunk (small final chunks
NCHUNK = len(CHS)             # shorten the end-of-kernel drain)
NQ = 4                        # src quarter tables (interleaved: q = src % 4)
TQ = NPAD // NQ               # 25600 rows per quarter table
XCH = 10                      # buckets per phase-A x chunk
NPRE = 384                    # one-hot pool depth (prebuild during AllGather)

_CACHE = {}
_PREP_CACHE = {}


def _prep(edge_index):
    """Host-side partitioning (data movement / index bookkeeping only)."""
    src = edge_index[0].astype(np.int64)
    dst = edge_index[1].astype(np.int64)

    loops = np.arange(N, dtype=np.int64)
    src2 = np.concatenate([src, loops])
    dst2 = np.concatenate([dst, loops])
    core = dst2 // SHARD
    l = dst2 % SHARD
    gl = l // GW
    slot = l % GW

    # norm = deg^-0.5 from out-degree (incl self-loops); 0 for padding nodes
    deg = np.bincount(src2, minlength=NPAD).astype(np.float64)
    with np.errstate(divide="ignore"):
        norm = np.where(deg > 0, 1.0 / np.sqrt(deg), 0.0).astype(np.float32)
    nrm = norm.reshape(NCORES, BPC, P).transpose(0, 2, 1)  # [c, p, b]
    nrm = np.ascontiguousarray(nrm)

    # ---- own-core stream: up to 128 same-core-source edges per dst group,
    # gathered from g_own during the AllGather (one tile per group) ----
    score = src2 // SHARD
    is_own = score == core
    ocell = core * NG + gl
    oorder = np.lexsort((src2, np.where(is_own, ocell, 2**40)))
    n_own_all = int(is_own.sum())
    oo = oorder[:n_own_all]                      # own edges sorted by ocell
    oc_s = ocell[oo]
    ocounts = np.bincount(oc_s, minlength=NCORES * NG)
    ostarts = np.zeros(NCORES * NG + 1, np.int64)
    np.cumsum(ocounts, out=ostarts[1:])
    opos = np.arange(n_own_all) - ostarts[oc_s]
    keep = opos < P                              # first 128 per group
    okeep = oo[keep]
    oprt = opos[keep]
    og = gl[okeep]
    ocorek = core[okeep]
    olidx = (src2[okeep] % SHARD).astype(np.int16)
    # own idx stream: 10 chunks x 10 tiles, 16-row wrap, replicated x8
    o16 = np.zeros((NCORES, 16, NG * 8), np.int16)
    OCH = 10                                     # own groups per gather
    oib = (og % OCH) * P + oprt
    ocol16 = (og // OCH) * OCH * 8 + oib // 16
    orow16 = oib % 16
    o16[ocorek, orow16, ocol16] = olidx
    o16 = np.tile(o16, (1, 8, 1))
    odst = np.full((NCORES, P, NG), 999.0, np.float32)
    odst[ocorek, oprt, og] = slot[okeep]

    # ---- remote stream: everything not claimed by the own stream ----
    claimed = np.zeros(len(src2), bool)
    claimed[okeep] = True
    rmask = ~claimed
    src2 = src2[rmask]
    dst2 = dst2[rmask]
    core = core[rmask]
    gl = gl[rmask]
    slot = slot[rmask]
    q = src2 % NQ
    lidx = src2 // NQ

    cell = (core * NG + gl) * NQ + q
    order = np.lexsort((lidx, cell))
    cell_s = cell[order]
    counts = np.bincount(cell, minlength=NCORES * NG * NQ)
    starts = np.zeros(NCORES * NG * NQ + 1, np.int64)
    np.cumsum(counts, out=starts[1:])
    pos = np.arange(len(order)) - starts[cell_s]

    caps = np.ceil(
        counts.reshape(NCORES, NG, NQ).max(0) / P
    ).astype(np.int64)                                     # [NG, NQ]

    # single stream layout: (chunk, q, gl-in-chunk, t)
    ch0 = np.cumsum([0] + CHS)
    chunk_of = np.repeat(np.arange(NCHUNK), CHS)
    tbase = np.zeros((NG, NQ), np.int64)
    blockstart = np.zeros((NCHUNK, NQ), np.int64)
    blockntiles = np.zeros((NCHUNK, NQ), np.int64)
    tc = 0
    for ch in range(NCHUNK):
        for qq in range(NQ):
            blockstart[ch, qq] = tc
            for g in range(ch0[ch], ch0[ch + 1]):
                tbase[g, qq] = tc
                tc += caps[g, qq]
            blockntiles[ch, qq] = tc - blockstart[ch, qq]
    totE = int(tc)

    ecore = core[order]
    egl = gl[order]
    eq = q[order]
    t = pos // P
    prt = pos % P

    gcol = tbase[egl, eq] + t
    chnk = chunk_of[egl]
    ib = (gcol - blockstart[chnk, eq]) * P + prt
    col16 = blockstart[chnk, eq] * 8 + ib // 16
    row16 = ib % 16
    e16 = np.zeros((NCORES, 16, totE * 8), np.int16)
    e16[ecore, row16, col16] = lidx[order].astype(np.int16)
    e16 = np.tile(e16, (1, 8, 1))

    edst = np.full((NCORES, P, totE), 999.0, np.float32)
    edst[ecore, prt, gcol] = slot[order]

    return dict(
        e16=e16, edst=edst, nrm=nrm, o16=o16, odst=odst,
        caps=caps, tbase=tbase, blockstart=blockstart,
        blockntiles=blockntiles, totE=totE,
    )


def _build(prep):
    import concourse.bass as bass
    import concourse.bacc as bacc
    import concourse.mybir as mybir
    import concourse.tile as tile

    F32 = mybir.dt.float32
    BF16 = mybir.dt.bfloat16
    F8 = mybir.dt.float8e4
    I16 = mybir.dt.int16
    AF = mybir.ActivationFunctionType
    OP = mybir.AluOpType

    caps = prep["caps"]
    tbase = prep["tbase"]
    blockstart = prep["blockstart"]
    blockntiles = prep["blockntiles"]
    totE = prep["totE"]
    maxnt = int(blockntiles.max())
    CH0 = [0]
    for n_ in CHS:
        CH0.append(CH0[-1] + n_)

    nc = bacc.Bacc("TRN2", target_bir_lowering=False, debug=False)
    x_sh = nc.dram_tensor("x_sh", [P, BPC * D], BF16, kind="ExternalInput")
    w_in = nc.dram_tensor("w_in", [D, D], F32, kind="ExternalInput")
    iota_in = nc.dram_tensor("iota_in", [P, P], BF16, kind="ExternalInput")
    nrm_in = nc.dram_tensor("nrm_in", [P, BPC], F32, kind="ExternalInput")
    e16_in = nc.dram_tensor("e16_in", [P, totE * 8], I16, kind="ExternalInput")
    edst_in = nc.dram_tensor("edst_in", [P, totE], F32, kind="ExternalInput")
    o16_in = nc.dram_tensor("o16_in", [P, NG * 8], I16, kind="ExternalInput")
    odst_in = nc.dram_tensor("odst_in", [P, NG], F32, kind="ExternalInput")
    out = nc.dram_tensor("out", [SHARD, D], F32, kind="ExternalOutput")

    with tile.TileContext(nc) as tc:
        with (
            tc.tile_pool(name="const", bufs=1) as cst,
            tc.tile_pool(name="inp", bufs=1) as inp,
            tc.tile_pool(name="xp", bufs=6) as xp,
            tc.tile_pool(name="gp", bufs=4) as gp,
            tc.tile_pool(name="ohp", bufs=NPRE) as ohp,
            tc.tile_pool(name="oohp", bufs=NG) as oohp,
            tc.tile_pool(name="xgp", bufs=12) as xgp,
            tc.tile_pool(name="atp", bufs=8) as atp,
            tc.tile_pool(name="sqp", bufs=6) as sqp,
            tc.tile_pool(name="stfp", bufs=6) as stfp,
            tc.tile_pool(name="stage", bufs=1) as stg,
            tc.tile_pool(name="pagg", bufs=6, space="PSUM") as pap,
            tc.tile_pool(name="pw", bufs=2, space="PSUM") as pwp,
            tc.tile_pool(name="dram", bufs=1, space="DRAM") as drm,
        ):
            # ---- constants (phase-A-critical DMAs first) ----
            iota_t = cst.tile([P, P], BF16)
            w_sb = cst.tile([P, P], F32)
            w_bf = cst.tile([P, P], BF16)
            eps_t = cst.tile([P, 1], F32)
            nrm_t = inp.tile([P, BPC], F32)
            nc.sync.dma_start(out=nrm_t[:], in_=nrm_in[:])
            nc.gpsimd.memset(eps_t[:], 1e-30)

            # ---- staging ----
            out_stage = stg.tile([P, BPC * P], BF16)
            ssq = stg.tile([P, BPC], F32)
            rl2 = stg.tile([P, BPC], F32)

            g_own = drm.tile([SHARD, D], BF16)
            g_full = drm.tile([NPAD, D], BF16, addr_space="Shared")

            x_r = x_sh[:].rearrange("p (b f) -> p b f", f=D)
            gown_r = g_own[:].rearrange("(b p) f -> p b f", p=P)

            # ---- phase A: g = norm[src] * x (bf16) ----
            for xc in range(BPC // XCH):
                sl = slice(xc * XCH, (xc + 1) * XCH)
                eng = nc.sync if xc % 2 == 0 else nc.scalar
                xch = xp.tile([P, XCH, P], BF16, tag="xch")
                eng.dma_start(out=xch[:], in_=x_r[:, sl, :])
                gch = gp.tile([P, XCH, P], BF16, tag="gch")
                meng = nc.gpsimd if xc % 2 == 0 else nc.vector
                meng.tensor_tensor(
                    out=gch[:], in0=xch[:],
                    in1=nrm_t[:, sl].rearrange("p b -> p b ()")
                        .to_broadcast([P, XCH, P]),
                    op=OP.mult,
                )
                eng2 = nc.scalar if xc % 2 == 0 else nc.sync
                eng2.dma_start(out=gown_r[:, sl, :], in_=gch[:])

            nc.gpsimd.collective_compute(
                "AllGather",
                mybir.AluOpType.bypass,
                ins=[g_own.opt()],
                outs=[g_full.opt()],
                replica_groups=[list(range(NCORES))],
            )

            # ---- remaining input streams (load during the AllGather) ----
            e16_t = inp.tile([P, totE * 8], I16)
            edst_t = inp.tile([P, totE], F32)
            o16_t = inp.tile([P, NG * 8], I16)
            odst_t = inp.tile([P, NG], F32)
            nc.sync.dma_start(out=iota_t[:], in_=iota_in[:])
            nc.sync.dma_start(out=edst_t[:], in_=edst_in[:])
            nc.sync.dma_start(out=o16_t[:], in_=o16_in[:])
            nc.sync.dma_start(out=odst_t[:], in_=odst_in[:])

            # ---- one-hot builds (fp8; DVE runs ahead during the AllGather) ----
            ohs = {}
            for ch in range(NCHUNK):
                for qq in range(NQ):
                    for g in range(CH0[ch], CH0[ch + 1]):
                        for t in range(int(caps[g, qq])):
                            col = int(tbase[g, qq]) + t
                            oh = ohp.tile([P, P], F8, tag="oh")
                            nc.vector.tensor_scalar(
                                out=oh[:], in0=iota_t[:],
                                scalar1=edst_t[:, col:col + 1], scalar2=None,
                                op0=OP.is_equal,
                            )
                            ohs[(g, qq, t)] = oh

            # ---- own-core stream: one-hots + gathers from g_own run during
            # the AllGather (no dependency on g_full) ----
            oohs = []
            for g in range(NG):
                ooh = oohp.tile([P, P], F8, tag="ooh")
                nc.vector.tensor_scalar(
                    out=ooh[:], in0=iota_t[:],
                    scalar1=odst_t[:, g:g + 1], scalar2=None,
                    op0=OP.is_equal,
                )
                oohs.append(ooh)
            xo_t = stg.tile([P, NG, P], BF16)
            OCH = 10
            for och in range(NG // OCH):
                nc.gpsimd.dma_gather(
                    out_ap=xo_t[:, och * OCH:(och + 1) * OCH, :],
                    in_ap=g_own[:],
                    idxs_ap=o16_t[:, och * OCH * 8:(och + 1) * OCH * 8],
                    num_idxs=OCH * P,
                    num_idxs_reg=OCH * P,
                    elem_size=P,
                    single_packet=False,
                )

            # e16/w deferred past phase A: load during the AllGather without
            # stealing phase-A DMA bandwidth
            with tc.tile_wait_until(0.09):
                nc.sync.dma_start(out=e16_t[:], in_=e16_in[:])
                nc.sync.dma_start(out=w_sb[:], in_=w_in[:])
                nc.vector.tensor_copy(w_bf[:], w_sb[:])

            # ---- phase C: gather + scatter-sum + @W + stage ----
            # tail parts fire as soon as their groups' ssq is complete
            TAILS = [(0, 40), (40, 64), (64, 84)]
            out_r = out[:].rearrange("(b p) f -> p b f", p=P)

            def tail_part(g0, g1):
                """rl2 + tanh + store for groups [g0, g1)."""
                sl2 = slice(g0, g1)
                nc.scalar.activation(rl2[:, sl2], ssq[:, sl2], AF.Ln,
                                     bias=eps_t[:])
                nc.scalar.activation(rl2[:, sl2], rl2[:, sl2], AF.Exp,
                                     scale=-0.5)
                for b0 in range(g0, g1, 4):
                    stf = stfp.tile([P, 4, P], F32, tag="stf")
                    for j in range(4):
                        g = b0 + j
                        nc.scalar.activation(
                            stf[:, j, :], out_stage[:, g * P:(g + 1) * P],
                            AF.Tanh, scale=rl2[:, g:g + 1],
                        )
                    nc.sync.dma_start(
                        out=out_r[:, b0:b0 + 4, :], in_=stf[:],
                    )

            for ch in range(NCHUNK):
                xgs = []
                for qq in range(NQ):
                    nt = int(blockntiles[ch, qq])
                    if nt == 0:
                        xgs.append(None)
                        continue
                    bs = int(blockstart[ch, qq])
                    xg = xgp.tile([P, maxnt, P], BF16, tag="xg")
                    gq = g_full[:].rearrange("(r s) f -> s r f", s=NQ)[qq]
                    nc.gpsimd.dma_gather(
                        out_ap=xg[:, :nt, :],
                        in_ap=gq,
                        idxs_ap=e16_t[:, bs * 8:(bs + nt) * 8],
                        num_idxs=nt * P,
                        num_idxs_reg=nt * P,
                        elem_size=P,
                        elem_step=NQ * P,
                        single_packet=False,
                    )
                    xgs.append(xg)
                for g in range(CH0[ch], CH0[ch + 1]):
                    nmm = int(caps[g, :].sum()) + 1
                    pA = pap.tile([P, P], F32, space="PSUM")
                    nc.tensor.matmul(
                        pA[:], lhsT=xo_t[:, g, :], rhs=oohs[g][:],
                        start=True, stop=(nmm == 1),
                    )
                    mi = 1
                    for qq in range(NQ):
                        kq = int(caps[g, qq])
                        if kq == 0:
                            continue
                        p0 = int(tbase[g, qq]) - int(blockstart[ch, qq])
                        for t in range(kq):
                            nc.tensor.matmul(
                                pA[:],
                                lhsT=xgs[qq][:, p0 + t, :],
                                rhs=ohs.pop((g, qq, t))[:],
                                start=(mi == 0), stop=(mi == nmm - 1),
                            )
                            mi += 1
                    at = atp.tile([P, P], BF16, tag="at")
                    nc.vector.tensor_copy(at[:], pA[:])
                    pC = pwp.tile([P, P], F32, space="PSUM")
                    nc.tensor.matmul(
                        pC[:], lhsT=at[:], rhs=w_bf[:], start=True, stop=True,
                    )
                    ost = out_stage[:, g * P:(g + 1) * P]
                    nc.scalar.activation(ost, pC[:], AF.Copy)
                    sq = sqp.tile([P, P], BF16, tag="sq")
                    nc.vector.tensor_tensor(
                        out=sq[:], in0=ost, in1=ost, op=OP.mult,
                    )
                    nc.vector.tensor_reduce(
                        out=ssq[:, g:g + 1],
                        in_=sq[:].rearrange("p (a b) -> p a b", a=1),
                        axis=mybir.AxisListType.X, op=OP.add,
                    )
                done = CH0[ch + 1]
                for t0, t1 in TAILS:
                    if CH0[ch] < t1 <= done:
                        tail_part(t0, t1)
            tail_part(TAILS[-1][1], NG)

    nc.compile()
    return nc


def _make_in_maps(x, W, prep):
    iota_row = np.tile(
        np.arange(P, dtype=np.float32), (P, 1)
    ).astype(ml_dtypes.bfloat16)
    x_pad = np.zeros((NPAD, D), ml_dtypes.bfloat16)
    x_pad[:N] = np.asarray(x, np.float32).astype(ml_dtypes.bfloat16)
    w_np = np.asarray(W, np.float32)
    in_maps = []
    for c in range(NCORES):
        in_maps.append({
            "x_sh": np.ascontiguousarray(
                x_pad[c * SHARD:(c + 1) * SHARD]
                .reshape(BPC, P, D).transpose(1, 0, 2).reshape(P, BPC * D)),
            "w_in": w_np,
            "iota_in": iota_row,
            "nrm_in": np.ascontiguousarray(prep["nrm"][c]),
            "e16_in": np.ascontiguousarray(prep["e16"][c]),
            "edst_in": np.ascontiguousarray(prep["edst"][c]),
            "o16_in": np.ascontiguousarray(prep["o16"][c]),
            "odst_in": np.ascontiguousarray(prep["odst"][c]),
        })
    return in_maps


def get_compiled(edge_index):
    ei = np.asarray(edge_index)
    pkey = hash(ei.tobytes())
    if pkey in _PREP_CACHE:
        prep = _PREP_CACHE[pkey]
    else:
        prep = _prep(ei)
        _PREP_CACHE.clear()
        _PREP_CACHE[pkey] = prep
    key = prep["caps"].tobytes()
    if key not in _CACHE:
        _CACHE[key] = _build(prep)
    return _CACHE[key], prep


def kernel(x, edge_index, W):
    from concourse.bass_utils import run_bass_kernel_spmd

    nc, prep = get_compiled(edge_index)
    in_maps = _make_in_maps(x, W, prep)
    res = run_bass_kernel_spmd(nc, in_maps, core_ids=list(range(NCORES)))
    big = np.concatenate([res.results[c]["out"] for c in range(NCORES)], axis=0)
    return np.ascontiguousarray(big[:N]).astype(np.float32)
